# revision 41
# baseline (speedup 1.0000x reference)
"""Banded multi-head attention on 8 Trainium2 NeuronCores.

Problem: B=2, L=2048, D=1024, H=16 heads, d_k=64. The band mask is a 0/1
FLOAT tensor *added* to the scores (not -inf masked), so the softmax is
dense; exp(s + band) = exp(s) * e^band is handled by multiplying constant
e-or-1 parallelogram tiles over the band region.

Sharding: (batch x 4-head-groups) across the 8 cores. Host pre-transposes
activations/weights so every device matmul has its contraction dim on
partitions; the per-core partial output projections are summed on the host
(gather/unshard) together with the output bias.

Datapath is bf16 on the PE (1 cycle/row vs 3 for fp32), fp32 in PSUM.
Phase D is software-pipelined: scores(i) interleave with PV(i-1) reading
stored pt tiles so the PE never waits on the exp stream; exp is split
between the Scalar engine (table exp) and the DVE (Schraudolph int16
bit-trick exp); band multiplies run on GpSimd. Softmax normalization:
reciprocal of the sums row via scalar exp(-ln(s)), broadcast over 64
partitions with a rank-1 PE matmul, multiplied in on the DVE.
"""

import sys

sys.path.insert(0, "/opt/trn_rl_repo")

import numpy as np
import ml_dtypes
from contextlib import ExitStack

import concourse.bass as bass
import concourse.tile as tile
from concourse import bacc, mybir
from concourse.bass_utils import run_bass_kernel_spmd

dt = mybir.dt
AF = mybir.ActivationFunctionType
ALU = mybir.AluOpType
bf16 = dt.float16
BF = np.float16

B, L, D, H, DK = 2, 2048, 1024, 16, 64
HPC = 4            # heads per core
HD = HPC * DK      # 256: head dims per core
NQC, QCW = 4, 512  # q chunks (phases B/C/E)
NQP, QPW = 2, 1024 # q chunks (phase D)
NKB, KBW = 16, 128 # k blocks
NDC, DCW = 8, 128  # D chunks
SCALE = 1.0 / 8.0  # 1/sqrt(d_k)

# Schraudolph exp in fp16 bit-space: int16(round(x*SCALE*1024*log2(e) + b))
# bitcast to fp16 ~= exp(x*SCALE), max rel err ~4% (linear-interp sawtooth).
SCH_A = float(1024.0 * np.log2(np.e) * SCALE)
SCH_B = 15360.0 - 59.6
DVE_KBS = (5, 13)  # k-blocks whose exp runs on the DVE
# act_info.json set index of natural_log_exp_and_others: holds exp+ln+copy so
# the scalar engine never reloads its activation table mid-kernel.
ACT_SET_EXP_LN = 6


def _act(engine, *args, **kw):
    bi = engine.activation(*args, **kw)
    try:
        bi.act_func_set_id = ACT_SET_EXP_LN
    except Exception:
        pass
    return bi

_CACHE = {}


def _band_slots(half):
    """delta -> (slot, c0, c1) for 128x512 tiles at k-offset kb*128, q-offset
    qc*512, delta = kb*128 - qc*512. Band cols: f in [delta-half, delta+127+half]."""
    slots = {}
    d = -((half + 127) // 128) * 128
    while d <= half + 511:
        c0, c1 = max(0, d - half), min(512, d + 128 + half)
        if c0 < c1:
            slots[d] = (len(slots), c0, c1)
        d += 128
    return slots


def _build(masksize):
    half = int(masksize) // 2
    slots = _band_slots(half)
    ns = max(len(slots), 1)

    nc = bacc.Bacc("TRN2", target_bir_lowering=False, debug=False)

    f32 = dt.float32
    i16 = dt.int16
    # x tensors packed on host as [128, NQC, NDC, 512] so each q-chunk is one
    # contiguous 1 MB DMA with 8 KB rows.
    xq = nc.dram_tensor("xq", [128, NQC * NDC * QCW], bf16, kind="ExternalInput").ap()
    xk = nc.dram_tensor("xk", [128, NQC * NDC * QCW], bf16, kind="ExternalInput").ap()
    xv = nc.dram_tensor("xv", [128, NQC * NDC * QCW], bf16, kind="ExternalInput").ap()
    wq = nc.dram_tensor("wq", [128, NDC * HD], bf16, kind="ExternalInput").ap()
    wk = nc.dram_tensor("wk", [128, NDC * HD], bf16, kind="ExternalInput").ap()
    wv = nc.dram_tensor("wv", [128, NDC * HD], bf16, kind="ExternalInput").ap()
    wo = nc.dram_tensor("wo", [128, 2 * D], bf16, kind="ExternalInput").ap()
    bq = nc.dram_tensor("bq", [128, 2], f32, kind="ExternalInput").ap()
    bk = nc.dram_tensor("bk", [128, 4], f32, kind="ExternalInput").ap()
    bv = nc.dram_tensor("bv", [128, HD + 2], f32, kind="ExternalInput").ap()
    em = nc.dram_tensor("em", [128, ns * 512], bf16, kind="ExternalInput").ap()
    on1 = nc.dram_tensor("on1", [128, 64], bf16, kind="ExternalInput").ap()
    yt = nc.dram_tensor("yt", [D, L], bf16, kind="ExternalOutput").ap()

    with tile.TileContext(nc) as tc, ExitStack() as ctx:
        ctx.enter_context(
            nc.allow_low_precision(reason="bf16 matmul operands are intentional")
        )
        # ---- persistent SBUF ----
        wts = ctx.enter_context(tc.tile_pool(name="wts", bufs=1))
        big = ctx.enter_context(tc.tile_pool(name="big", bufs=1))

        wq_sb = wts.tile([128, NDC * HD], bf16, tag="wq", name="wq")
        wk_sb = wts.tile([128, NDC * HD], bf16, tag="wk", name="wk")
        wv_sb = wts.tile([128, NDC * HD], bf16, tag="wv", name="wv")
        wo_sb = wts.tile([128, 2 * D], bf16, tag="wo", name="wo")
        bq_sb = wts.tile([128, 2], f32, tag="bq", name="bq")
        bk_sb = wts.tile([128, 4], f32, tag="bk", name="bk")
        bv_sb = wts.tile([128, HD + 2], f32, tag="bv", name="bv")
        em_sb = wts.tile([128, ns * 512], bf16, tag="em", name="em")
        on1_sb = wts.tile([128, 64], bf16, tag="on1", name="on1")
        for t_sb, t_in in ((wq_sb, wq), (wk_sb, wk), (bq_sb, bq), (bk_sb, bk),
                           (wv_sb, wv), (bv_sb, bv), (wo_sb, wo), (em_sb, em),
                           (on1_sb, on1)):
            nc.sync.dma_start(t_sb[:], t_in[:])

        qt_sb = [big.tile([128, L], bf16, tag=f"qt{t}", name=f"qt{t}") for t in range(2)]
        # per-head k tiles, zero-padded so every scores matmul contracts over
        # the full 128 partitions (mixed 64/128-row PE configs serialize the
        # array): head h occupies rows (h%2)*64..+64, the other half is 0.
        kt_sb = [big.tile([128, L], bf16, tag=f"kt{h}", name=f"kt{h}") for h in range(HPC)]
        # attention outputs packed as head PAIRS [128, L] so the output
        # projection contracts over 128 rows in one matmul per pair.
        ot_sb = [big.tile([128, L], bf16, tag=f"ot{tp}", name=f"ot{tp}") for tp in range(2)]
        vaug_sb = [big.tile([128, HPC * 66], bf16, tag=f"vaug{lb}", name=f"vaug{lb}")
                   for lb in range(NKB)]
        for h in range(HPC):
            zlo = 64 if h % 2 == 0 else 0
            nc.gpsimd.memset(kt_sb[h][zlo:zlo + 64, :], 0.0)

        # ---- phase B: q/k projections (T-layout) ----
        with tc.tile_pool(name="xs", bufs=3) as xs, \
             tc.tile_pool(name="pqk", bufs=2, space="PSUM") as pqk:
            for qc in range(NQC):
                xq_t = xs.tile([128, NDC * QCW], bf16, tag="xq", name="xq")
                nc.sync.dma_start(
                    xq_t[:], xq[:, qc * NDC * QCW:(qc + 1) * NDC * QCW]
                )
                xk_t = xs.tile([128, NDC * QCW], bf16, tag="xk", name="xk")
                nc.sync.dma_start(
                    xk_t[:], xk[:, qc * NDC * QCW:(qc + 1) * NDC * QCW]
                )
                pq = [pqk.tile([128, QCW], f32, tag=f"pq{t}", name=f"pq{t}") for t in range(2)]
                pk = [pqk.tile([128, QCW], f32, tag=f"pk{t}", name=f"pk{t}") for t in range(2)]
                for c in range(NDC):
                    for t in range(2):
                        nc.tensor.matmul(
                            pq[t][:], wq_sb[:, c * HD + t * 128: c * HD + (t + 1) * 128],
                            xq_t[:, c * QCW:(c + 1) * QCW],
                            start=(c == 0), stop=(c == NDC - 1),
                        )
                        nc.tensor.matmul(
                            pk[t][:], wk_sb[:, c * HD + t * 128: c * HD + (t + 1) * 128],
                            xk_t[:, c * QCW:(c + 1) * QCW],
                            start=(c == 0), stop=(c == NDC - 1),
                        )
                for t in range(2):
                    nc.vector.tensor_scalar_add(
                        qt_sb[t][:, qc * QCW:(qc + 1) * QCW], pq[t][:], bq_sb[:, t:t + 1]
                    )
                    for hh in range(2):
                        h = 2 * t + hh
                        po = hh * 64
                        nc.vector.tensor_scalar_add(
                            kt_sb[h][po:po + 64, qc * QCW:(qc + 1) * QCW],
                            pk[t][po:po + 64, :], bk_sb[po:po + 64, h:h + 1]
                        )

        # ---- phase C: v in natural [L, HD] layout, +bias, +ones col ----
        with tc.tile_pool(name="xvp", bufs=2) as xvs, \
             tc.tile_pool(name="pvp", bufs=2, space="PSUM") as pvp:
            for lg in range(4):
                xv_t = xvs.tile([128, NDC * QCW], bf16, tag="xv", name="xv")
                nc.sync.dma_start(
                    xv_t[:], xv[:, lg * NDC * QCW:(lg + 1) * NDC * QCW]
                )
                pv = [pvp.tile([128, HD], f32, tag=f"pv{j}", name=f"pv{j}") for j in range(4)]
                for c in range(NDC):
                    for j in range(4):
                        nc.tensor.matmul(
                            pv[j][:], xv_t[:, c * QCW + j * 128: c * QCW + (j + 1) * 128],
                            wv_sb[:, c * HD:(c + 1) * HD],
                            start=(c == 0), stop=(c == NDC - 1),
                        )
                for j in range(4):
                    lb = lg * 4 + j
                    for h in range(HPC):
                        nc.vector.tensor_add(
                            vaug_sb[lb][:, h * 66: h * 66 + 64],
                            pv[j][:, h * DK:(h + 1) * DK],
                            bv_sb[:, h * DK:(h + 1) * DK],
                        )
                        nc.gpsimd.tensor_copy(
                            vaug_sb[lb][:, h * 66 + 64: h * 66 + 66],
                            bv_sb[:, HD:HD + 2],
                        )

        # ---- phase D: software-pipelined attention ----
        NIT = HPC * NQP  # 8 super-iterations of (head, q-half)
        with tc.tile_pool(name="psp", bufs=2, space="PSUM") as psp, \
             tc.tile_pool(name="pop", bufs=1, space="PSUM") as pop, \
             tc.tile_pool(name="bcp", bufs=2, space="PSUM") as bcp, \
             tc.tile_pool(name="ptp", bufs=34) as ptp, \
             tc.tile_pool(name="rcp", bufs=4) as rcp:
            pt_hist = {}     # i -> [16 pt tiles]
            pouts_hist = {}  # i -> [2 pouts tiles]

            # zero the 4 rotating rre buffers once; afterwards only row 0 is
            # ever written, and the 128-row broadcast matmul multiplies rows
            # 1-127 against zero weights.
            for _ in range(4):
                rre0 = rcp.tile([128, QCW], bf16, tag="rre", name="rre")
                nc.gpsimd.memset(rre0[:], 0.0)

            def emit_scores_exp(i):
                h, qp = divmod(i, NQP)
                t = h // 2
                ps = psp.tile([128, QPW], f32, tag="ps", name="ps")
                kb = emit_scores_exp.kb
                for j in range(2):
                    nc.tensor.matmul(
                        ps[:, j * QCW:(j + 1) * QCW],
                        kt_sb[h][:, kb * KBW:(kb + 1) * KBW],
                        qt_sb[t][:, qp * QPW + j * QCW: qp * QPW + (j + 1) * QCW],
                        start=True, stop=True,
                    )
                pt = ptp.tile([128, QPW], i16, tag="pt", name="pt")
                pt_hist[i].append(pt)
                if kb in DVE_KBS:
                    nc.vector.tensor_scalar(
                        pt[:], ps[:], SCH_A, SCH_B, op0=ALU.mult, op1=ALU.add,
                    )
                else:
                    _act(nc.scalar, pt[:].bitcast(bf16), ps[:], AF.Exp, scale=SCALE)
                for j in range(2):
                    qc = qp * 2 + j
                    delta = kb * KBW - qc * QCW
                    if delta in slots:
                        si, c0, c1 = slots[delta]
                        nc.gpsimd.tensor_mul(
                            pt[:].bitcast(bf16)[:, j * QCW + c0: j * QCW + c1],
                            pt[:].bitcast(bf16)[:, j * QCW + c0: j * QCW + c1],
                            em_sb[:, si * 512 + c0: si * 512 + c1],
                        )

            def emit_pv(i):
                h, qp = divmod(i, NQP)
                kb = emit_pv.kb
                if kb == 0:
                    pouts_hist[i] = [
                        pop.tile([66, QCW], f32, tag=f"pout{j}", name=f"pout{j}")
                        for j in range(2)
                    ]
                pouts = pouts_hist[i]
                pt = pt_hist[i][kb]
                for j in range(2):
                    nc.tensor.matmul(
                        pouts[j][:],
                        vaug_sb[kb][:, h * 66:(h + 1) * 66],
                        pt[:].bitcast(bf16)[:, j * QCW:(j + 1) * QCW],
                        start=(kb == 0), stop=(kb == NKB - 1),
                    )

            def emit_norm(i):
                h, qp = divmod(i, NQP)
                tp, po = h // 2, (h % 2) * 64
                pouts = pouts_hist.pop(i)
                recs = []
                for j in range(2):
                    ssum = rcp.tile([1, QCW], f32, tag="ssum", name="ssum")
                    nc.vector.tensor_copy(ssum[:], pouts[j][64:65, :])
                    recf = rcp.tile([1, QCW], f32, tag="recf", name="recf")
                    nc.vector.reciprocal_approx_fast(out=recf[:], in_=ssum[:])
                    recs.append(recf)
                rres = []
                for j in range(2):
                    rre = rcp.tile([128, QCW], bf16, tag="rre", name="rre")
                    nc.gpsimd.tensor_copy(rre[0:1, :], recs[j][:])
                    rres.append(rre)
                for j in range(2):
                    qc = qp * 2 + j
                    bc = bcp.tile([64, QCW], f32, tag="bc", name="bc")
                    nc.tensor.matmul(bc[:], on1_sb[:], rres[j][:], start=True, stop=True)
                    bc_sb = rcp.tile([64, QCW], bf16, tag="bcs", name="bcs")
                    nc.scalar.copy(bc_sb[:], bc[:])
                    nc.vector.tensor_mul(
                        ot_sb[tp][po:po + 64, qc * QCW:(qc + 1) * QCW],
                        pouts[j][0:64, :], bc_sb[:],
                    )
                del pt_hist[i]

            # pipeline: emission slot i runs scores(i) + pv(i-1) + norm(i-1)
            for i in range(NIT + 1):
                if i <= NIT - 1:
                    pt_hist[i] = []
                for kb in range(NKB):
                    if i <= NIT - 1:
                        emit_scores_exp.kb = kb
                        emit_scores_exp(i)
                    if i >= 1 and kb >= 1:
                        emit_pv.kb = kb - 1
                        emit_pv(i - 1)
                if i >= 1:
                    emit_pv.kb = NKB - 1
                    emit_pv(i - 1)
                    emit_norm(i - 1)

        # ---- phase E: output projection yT partial ----
        with tc.tile_pool(name="ysp", bufs=3) as ysp, \
             tc.tile_pool(name="pyp", bufs=2, space="PSUM") as pyp:
            for db in range(NDC):
                py = pyp.tile([128, NQC * QCW], f32, tag="py", name="py")  # 4 banks
                for tp in range(2):
                    for qc in range(NQC):
                        nc.tensor.matmul(
                            py[:, qc * QCW:(qc + 1) * QCW],
                            wo_sb[:, tp * D + db * DCW: tp * D + (db + 1) * DCW],
                            ot_sb[tp][:, qc * QCW:(qc + 1) * QCW],
                            start=(tp == 0), stop=(tp == 1),
                        )
                y_sb = ysp.tile([128, NQC * QCW], bf16, tag="y", name="y")
                half = NQC * QCW // 2
                nc.vector.tensor_copy(y_sb[:, 0:half], py[:, 0:half])
                _act(nc.scalar, y_sb[:, half:], py[:, half:], AF.Copy)
                nc.sync.dma_start(yt[db * DCW:(db + 1) * DCW, :], y_sb[:])

    nc.compile()
    return nc


def _pack_ndc(w_g):
    """[HD, D] row-slice of a Linear weight -> [128, NDC*HD] SBUF image with
    w[p, c*HD+n] = w_g[n, c*128+p] (lhsT chunks along the free dim)."""
    return np.ascontiguousarray(
        w_g.reshape(HD, NDC, 128).transpose(2, 1, 0).reshape(128, NDC * HD)
    )


def _pack_x(xT):
    """[D, L] activation-transpose -> [128, NQC*NDC*512] with block (qc, c) =
    xT[c*128:(c+1)*128, qc*512:(qc+1)*512], so phase B/C DMAs are contiguous."""
    # xT [D=NDC*128, L=NQC*512] -> [NDC, 128, NQC, 512] -> [128, NQC, NDC, 512]
    return np.ascontiguousarray(
        xT.reshape(NDC, 128, NQC, QCW).transpose(1, 2, 0, 3).reshape(128, NQC * NDC * QCW)
    ).astype(BF)


def _prep_inmaps(query, key, value, Wq, bq, Wk, bk, Wv, bv, Wo, masksize):
    half = int(masksize) // 2
    slots = _band_slots(half)
    ns = max(len(slots), 1)
    em = np.ones((128, ns * 512), np.float32)
    e1 = np.float32(np.exp(np.float32(1.0)))
    p = np.arange(128)[:, None]
    f = np.arange(512)[None, :]
    for d, (si, _, _) in slots.items():
        em[:, si * 512:(si + 1) * 512] = np.where(
            np.abs(d + p - f) <= half, e1, np.float32(1.0)
        )
    em = em.astype(BF)

    xqP = [_pack_x(np.ascontiguousarray(query[b].T)) for b in range(B)]
    xkP = [_pack_x(np.ascontiguousarray(key[b].T)) for b in range(B)]
    xvP = [_pack_x(np.ascontiguousarray(value[b].T)) for b in range(B)]
    wqP = [_pack_ndc(Wq[g * HD:(g + 1) * HD, :]).astype(BF) for g in range(4)]
    wkP = [_pack_ndc(Wk[g * HD:(g + 1) * HD, :]).astype(BF) for g in range(4)]
    wvP = [_pack_ndc(Wv[g * HD:(g + 1) * HD, :]).astype(BF) for g in range(4)]
    # wo: per head-PAIR tp, [128, D] = Wo[:, g*HD+tp*128 : +128].T, laid side
    # by side -> [128, 2*D]
    woP = [
        np.ascontiguousarray(
            np.concatenate(
                [Wo[:, g * HD + tp * 128: g * HD + (tp + 1) * 128].T for tp in range(2)],
                axis=1,
            )
        ).astype(BF)
        for g in range(4)
    ]
    bqP = [np.ascontiguousarray(bq[g * HD:(g + 1) * HD].reshape(2, 128).T) for g in range(4)]
    # bk: column h holds head h's bias in rows (h%2)*64..+64 (rest unused)
    bkP = []
    for g in range(4):
        bz = np.zeros((128, 4), np.float32)
        for h in range(HPC):
            po = (h % 2) * 64
            bz[po:po + 64, h] = bk[g * HD + h * 64: g * HD + (h + 1) * 64]
        bkP.append(bz)
    bvP = [
        np.ascontiguousarray(
            np.concatenate(
                [np.tile(bv[g * HD:(g + 1) * HD], (128, 1)), np.ones((128, 2), np.float32)],
                axis=1,
            )
        )
        for g in range(4)
    ]

    on1z = np.zeros((128, 64), BF)
    on1z[0, :] = 1.0

    in_maps = []
    for c in range(8):
        b, g = c // 4, c % 4
        in_maps.append({
            "xq": xqP[b], "xk": xkP[b], "xv": xvP[b],
            "wq": wqP[g], "wk": wkP[g], "wv": wvP[g], "wo": woP[g],
            "bq": bqP[g], "bk": bkP[g], "bv": bvP[g], "em": em,
            "on1": on1z,
        })
    return in_maps


def kernel(query, key, value, Wq, bq, Wk, bk, Wv, bv, Wo, bo, masksize):
    query = np.asarray(query, dtype=np.float32)
    key = np.asarray(key, dtype=np.float32)
    value = np.asarray(value, dtype=np.float32)
    Wq, bq = np.asarray(Wq, np.float32), np.asarray(bq, np.float32)
    Wk, bk = np.asarray(Wk, np.float32), np.asarray(bk, np.float32)
    Wv, bv = np.asarray(Wv, np.float32), np.asarray(bv, np.float32)
    Wo, bo = np.asarray(Wo, np.float32), np.asarray(bo, np.float32)
    ms = int(np.asarray(masksize))

    if ms not in _CACHE:
        _CACHE[ms] = _build(ms)
    nc = _CACHE[ms]

    in_maps = _prep_inmaps(query, key, value, Wq, bq, Wk, bk, Wv, bv, Wo, ms)
    res = run_bass_kernel_spmd(nc, in_maps, list(range(8)))

    out = np.empty((B, L, D), np.float32)
    for b in range(B):
        acc = res.results[4 * b]["yt"].astype(np.float32)
        for g in range(1, 4):
            acc = acc + res.results[4 * b + g]["yt"].astype(np.float32)
        out[b] = acc.T + bo
    return out


# revision 43
# speedup vs baseline: 1.0089x; 1.0089x over previous
"""Banded multi-head attention on 8 Trainium2 NeuronCores.

Problem: B=2, L=2048, D=1024, H=16 heads, d_k=64. The band mask is a 0/1
FLOAT tensor *added* to the scores (not -inf masked), so the softmax is
dense; exp(s + band) = exp(s) * e^band is handled by multiplying constant
e-or-1 parallelogram tiles over the band region.

Sharding: (batch x 4-head-groups) across the 8 cores. Host pre-transposes
activations/weights so every device matmul has its contraction dim on
partitions; the per-core partial output projections are summed on the host
(gather/unshard) together with the output bias.

Datapath is bf16 on the PE (1 cycle/row vs 3 for fp32), fp32 in PSUM.
Phase D is software-pipelined: scores(i) interleave with PV(i-1) reading
stored pt tiles so the PE never waits on the exp stream; exp is split
between the Scalar engine (table exp) and the DVE (Schraudolph int16
bit-trick exp); band multiplies run on GpSimd. Softmax normalization:
reciprocal of the sums row via scalar exp(-ln(s)), broadcast over 64
partitions with a rank-1 PE matmul, multiplied in on the DVE.
"""

import sys

sys.path.insert(0, "/opt/trn_rl_repo")

import numpy as np
import ml_dtypes
from contextlib import ExitStack

import concourse.bass as bass
import concourse.tile as tile
from concourse import bacc, mybir
from concourse.bass_utils import run_bass_kernel_spmd

dt = mybir.dt
AF = mybir.ActivationFunctionType
ALU = mybir.AluOpType
bf16 = dt.float16
BF = np.float16

B, L, D, H, DK = 2, 2048, 1024, 16, 64
HPC = 4            # heads per core
HD = HPC * DK      # 256: head dims per core
NQC, QCW = 4, 512  # q chunks (phases B/C/E)
NQP, QPW = 2, 1024 # q chunks (phase D)
NKB, KBW = 16, 128 # k blocks
NDC, DCW = 8, 128  # D chunks
SCALE = 1.0 / 8.0  # 1/sqrt(d_k)

# Schraudolph exp in fp16 bit-space: int16(round(x*SCALE*1024*log2(e) + b))
# bitcast to fp16 ~= exp(x*SCALE), max rel err ~4% (linear-interp sawtooth).
SCH_A = float(1024.0 * np.log2(np.e) * SCALE)
SCH_B = 15360.0 - 59.6
DVE_KBS = (5, 13)  # k-blocks whose exp runs on the DVE
# act_info.json set index of natural_log_exp_and_others: holds exp+ln+copy so
# the scalar engine never reloads its activation table mid-kernel.
ACT_SET_EXP_LN = 6


def _act(engine, *args, **kw):
    bi = engine.activation(*args, **kw)
    try:
        bi.act_func_set_id = ACT_SET_EXP_LN
    except Exception:
        pass
    return bi

_CACHE = {}


def _band_slots(half):
    """delta -> (slot, c0, c1) for 128x512 tiles at k-offset kb*128, q-offset
    qc*512, delta = kb*128 - qc*512. Band cols: f in [delta-half, delta+127+half]."""
    slots = {}
    d = -((half + 127) // 128) * 128
    while d <= half + 511:
        c0, c1 = max(0, d - half), min(512, d + 128 + half)
        if c0 < c1:
            slots[d] = (len(slots), c0, c1)
        d += 128
    return slots


def _build(masksize):
    half = int(masksize) // 2
    slots = _band_slots(half)
    ns = max(len(slots), 1)

    nc = bacc.Bacc("TRN2", target_bir_lowering=False, debug=False)

    f32 = dt.float32
    i16 = dt.int16
    # x tensors packed on host as [128, NQC, NDC, 512] so each q-chunk is one
    # contiguous 1 MB DMA with 8 KB rows.
    xq = nc.dram_tensor("xq", [128, NQC * NDC * QCW], bf16, kind="ExternalInput").ap()
    xk = nc.dram_tensor("xk", [128, NQC * NDC * QCW], bf16, kind="ExternalInput").ap()
    xv = nc.dram_tensor("xv", [128, NQC * NDC * QCW], bf16, kind="ExternalInput").ap()
    wq = nc.dram_tensor("wq", [128, NDC * HD], bf16, kind="ExternalInput").ap()
    wk = nc.dram_tensor("wk", [128, NDC * HD], bf16, kind="ExternalInput").ap()
    wv = nc.dram_tensor("wv", [128, NDC * HD], bf16, kind="ExternalInput").ap()
    wo = nc.dram_tensor("wo", [128, 2 * D], bf16, kind="ExternalInput").ap()
    bq = nc.dram_tensor("bq", [128, 2], f32, kind="ExternalInput").ap()
    bk = nc.dram_tensor("bk", [128, 4], f32, kind="ExternalInput").ap()
    bv = nc.dram_tensor("bv", [128, HD + 2], f32, kind="ExternalInput").ap()
    em = nc.dram_tensor("em", [128, ns * 512], bf16, kind="ExternalInput").ap()
    on1 = nc.dram_tensor("on1", [128, 64], bf16, kind="ExternalInput").ap()
    yt = nc.dram_tensor("yt", [D, L], bf16, kind="ExternalOutput").ap()

    with tile.TileContext(nc) as tc, ExitStack() as ctx:
        ctx.enter_context(
            nc.allow_low_precision(reason="bf16 matmul operands are intentional")
        )
        # ---- persistent SBUF ----
        wts = ctx.enter_context(tc.tile_pool(name="wts", bufs=1))
        big = ctx.enter_context(tc.tile_pool(name="big", bufs=1))

        wq_sb = wts.tile([128, NDC * HD], bf16, tag="wq", name="wq")
        wk_sb = wts.tile([128, NDC * HD], bf16, tag="wk", name="wk")
        wv_sb = wts.tile([128, NDC * HD], bf16, tag="wv", name="wv")
        wo_sb = wts.tile([128, 2 * D], bf16, tag="wo", name="wo")
        bq_sb = wts.tile([128, 2], f32, tag="bq", name="bq")
        bk_sb = wts.tile([128, 4], f32, tag="bk", name="bk")
        bv_sb = wts.tile([128, HD + 2], f32, tag="bv", name="bv")
        em_sb = wts.tile([128, ns * 512], bf16, tag="em", name="em")
        on1_sb = wts.tile([128, 64], bf16, tag="on1", name="on1")
        for t_sb, t_in in ((wq_sb, wq), (wk_sb, wk), (bq_sb, bq), (bk_sb, bk),
                           (wv_sb, wv), (bv_sb, bv), (wo_sb, wo), (em_sb, em),
                           (on1_sb, on1)):
            nc.sync.dma_start(t_sb[:], t_in[:])

        qt_sb = [big.tile([128, L], bf16, tag=f"qt{t}", name=f"qt{t}") for t in range(2)]
        # per-head k tiles, zero-padded so every scores matmul contracts over
        # the full 128 partitions (mixed 64/128-row PE configs serialize the
        # array): head h occupies rows (h%2)*64..+64, the other half is 0.
        kt_sb = [big.tile([128, L], bf16, tag=f"kt{h}", name=f"kt{h}") for h in range(HPC)]
        # attention outputs packed as head PAIRS [128, L] so the output
        # projection contracts over 128 rows in one matmul per pair.
        ot_sb = [big.tile([128, L], bf16, tag=f"ot{tp}", name=f"ot{tp}") for tp in range(2)]
        vaug_sb = [big.tile([128, HPC * 66], bf16, tag=f"vaug{lb}", name=f"vaug{lb}")
                   for lb in range(NKB)]
        for h in range(HPC):
            zlo = 64 if h % 2 == 0 else 0
            nc.gpsimd.memset(kt_sb[h][zlo:zlo + 64, :], 0.0)

        # ---- phase B: q/k projections (T-layout) ----
        with tc.tile_pool(name="xs", bufs=3) as xs, \
             tc.tile_pool(name="pqk", bufs=2, space="PSUM") as pqk:
            for qc in range(NQC):
                xq_t = xs.tile([128, NDC * QCW], bf16, tag="xq", name="xq")
                nc.sync.dma_start(
                    xq_t[:], xq[:, qc * NDC * QCW:(qc + 1) * NDC * QCW]
                )
                xk_t = xs.tile([128, NDC * QCW], bf16, tag="xk", name="xk")
                nc.sync.dma_start(
                    xk_t[:], xk[:, qc * NDC * QCW:(qc + 1) * NDC * QCW]
                )
                pq = [pqk.tile([128, QCW], f32, tag=f"pq{t}", name=f"pq{t}") for t in range(2)]
                pk = [pqk.tile([128, QCW], f32, tag=f"pk{t}", name=f"pk{t}") for t in range(2)]
                for c in range(NDC):
                    for t in range(2):
                        nc.tensor.matmul(
                            pq[t][:], wq_sb[:, c * HD + t * 128: c * HD + (t + 1) * 128],
                            xq_t[:, c * QCW:(c + 1) * QCW],
                            start=(c == 0), stop=(c == NDC - 1),
                        )
                        nc.tensor.matmul(
                            pk[t][:], wk_sb[:, c * HD + t * 128: c * HD + (t + 1) * 128],
                            xk_t[:, c * QCW:(c + 1) * QCW],
                            start=(c == 0), stop=(c == NDC - 1),
                        )
                for t in range(2):
                    nc.vector.tensor_scalar_add(
                        qt_sb[t][:, qc * QCW:(qc + 1) * QCW], pq[t][:], bq_sb[:, t:t + 1]
                    )
                    for hh in range(2):
                        h = 2 * t + hh
                        po = hh * 64
                        nc.vector.tensor_scalar_add(
                            kt_sb[h][po:po + 64, qc * QCW:(qc + 1) * QCW],
                            pk[t][po:po + 64, :], bk_sb[po:po + 64, h:h + 1]
                        )

        # ---- phase C: v in natural [L, HD] layout, +bias, +ones col ----
        with tc.tile_pool(name="xvp", bufs=3) as xvs, \
             tc.tile_pool(name="pvp", bufs=2, space="PSUM") as pvp:
            for lg in range(4):
                xv_t = xvs.tile([128, NDC * QCW], bf16, tag="xv", name="xv")
                nc.sync.dma_start(
                    xv_t[:], xv[:, lg * NDC * QCW:(lg + 1) * NDC * QCW]
                )
                pv = [pvp.tile([128, HD], f32, tag=f"pv{j}", name=f"pv{j}") for j in range(4)]
                for c in range(NDC):
                    for j in range(4):
                        nc.tensor.matmul(
                            pv[j][:], xv_t[:, c * QCW + j * 128: c * QCW + (j + 1) * 128],
                            wv_sb[:, c * HD:(c + 1) * HD],
                            start=(c == 0), stop=(c == NDC - 1),
                        )
                for j in range(4):
                    lb = lg * 4 + j
                    for h in range(HPC):
                        nc.vector.tensor_add(
                            vaug_sb[lb][:, h * 66: h * 66 + 64],
                            pv[j][:, h * DK:(h + 1) * DK],
                            bv_sb[:, h * DK:(h + 1) * DK],
                        )
                        nc.gpsimd.tensor_copy(
                            vaug_sb[lb][:, h * 66 + 64: h * 66 + 66],
                            bv_sb[:, HD:HD + 2],
                        )

        # ---- phase D: software-pipelined attention ----
        NIT = HPC * NQP  # 8 super-iterations of (head, q-half)
        with tc.tile_pool(name="psp", bufs=2, space="PSUM") as psp, \
             tc.tile_pool(name="pop", bufs=1, space="PSUM") as pop, \
             tc.tile_pool(name="bcp", bufs=2, space="PSUM") as bcp, \
             tc.tile_pool(name="ptp", bufs=34) as ptp, \
             tc.tile_pool(name="rcp", bufs=4) as rcp:
            pt_hist = {}     # i -> [16 pt tiles]
            pouts_hist = {}  # i -> [2 pouts tiles]

            # zero the 4 rotating rre buffers once; afterwards only row 0 is
            # ever written, and the 128-row broadcast matmul multiplies rows
            # 1-127 against zero weights.
            for _ in range(4):
                rre0 = rcp.tile([128, QCW], bf16, tag="rre", name="rre")
                nc.gpsimd.memset(rre0[:], 0.0)

            def emit_scores_exp(i):
                h, qp = divmod(i, NQP)
                t = h // 2
                ps = psp.tile([128, QPW], f32, tag="ps", name="ps")
                kb = emit_scores_exp.kb
                for j in range(2):
                    nc.tensor.matmul(
                        ps[:, j * QCW:(j + 1) * QCW],
                        kt_sb[h][:, kb * KBW:(kb + 1) * KBW],
                        qt_sb[t][:, qp * QPW + j * QCW: qp * QPW + (j + 1) * QCW],
                        start=True, stop=True,
                    )
                pt = ptp.tile([128, QPW], i16, tag="pt", name="pt")
                pt_hist[i].append(pt)
                if kb in DVE_KBS:
                    nc.vector.tensor_scalar(
                        pt[:], ps[:], SCH_A, SCH_B, op0=ALU.mult, op1=ALU.add,
                    )
                else:
                    _act(nc.scalar, pt[:].bitcast(bf16), ps[:], AF.Exp, scale=SCALE)
                for j in range(2):
                    qc = qp * 2 + j
                    delta = kb * KBW - qc * QCW
                    if delta in slots:
                        si, c0, c1 = slots[delta]
                        nc.gpsimd.tensor_mul(
                            pt[:].bitcast(bf16)[:, j * QCW + c0: j * QCW + c1],
                            pt[:].bitcast(bf16)[:, j * QCW + c0: j * QCW + c1],
                            em_sb[:, si * 512 + c0: si * 512 + c1],
                        )

            def emit_pv(i):
                h, qp = divmod(i, NQP)
                kb = emit_pv.kb
                if kb == 0:
                    pouts_hist[i] = [
                        pop.tile([66, QCW], f32, tag=f"pout{j}", name=f"pout{j}")
                        for j in range(2)
                    ]
                pouts = pouts_hist[i]
                pt = pt_hist[i][kb]
                for j in range(2):
                    nc.tensor.matmul(
                        pouts[j][:],
                        vaug_sb[kb][:, h * 66:(h + 1) * 66],
                        pt[:].bitcast(bf16)[:, j * QCW:(j + 1) * QCW],
                        start=(kb == 0), stop=(kb == NKB - 1),
                    )

            def emit_norm(i):
                h, qp = divmod(i, NQP)
                tp, po = h // 2, (h % 2) * 64
                pouts = pouts_hist.pop(i)
                recs = []
                for j in range(2):
                    ssum = rcp.tile([1, QCW], f32, tag="ssum", name="ssum")
                    nc.vector.tensor_copy(ssum[:], pouts[j][64:65, :])
                    recf = rcp.tile([1, QCW], f32, tag="recf", name="recf")
                    nc.vector.reciprocal_approx_fast(out=recf[:], in_=ssum[:])
                    recs.append(recf)
                rres = []
                for j in range(2):
                    rre = rcp.tile([128, QCW], bf16, tag="rre", name="rre")
                    nc.gpsimd.tensor_copy(rre[0:1, :], recs[j][:])
                    rres.append(rre)
                for j in range(2):
                    qc = qp * 2 + j
                    bc = bcp.tile([64, QCW], f32, tag="bc", name="bc")
                    nc.tensor.matmul(bc[:], on1_sb[:], rres[j][:], start=True, stop=True)
                    bc_sb = rcp.tile([64, QCW], bf16, tag="bcs", name="bcs")
                    nc.vector.tensor_copy(bc_sb[:], bc[:])
                    nc.vector.tensor_mul(
                        ot_sb[tp][po:po + 64, qc * QCW:(qc + 1) * QCW],
                        pouts[j][0:64, :], bc_sb[:],
                    )
                del pt_hist[i]

            # pipeline: emission slot i runs scores(i) + pv(i-1) + norm(i-1)
            for i in range(NIT + 1):
                if i <= NIT - 1:
                    pt_hist[i] = []
                for kb in range(NKB):
                    if i <= NIT - 1:
                        emit_scores_exp.kb = kb
                        emit_scores_exp(i)
                    if i >= 1:
                        emit_pv.kb = kb
                        emit_pv(i - 1)
                if i >= 1:
                    emit_norm(i - 1)

        # ---- phase E: output projection yT partial ----
        with tc.tile_pool(name="ysp", bufs=3) as ysp, \
             tc.tile_pool(name="pyp", bufs=2, space="PSUM") as pyp:
            for db in range(NDC):
                py = pyp.tile([128, NQC * QCW], f32, tag="py", name="py")  # 4 banks
                for tp in range(2):
                    for qc in range(NQC):
                        nc.tensor.matmul(
                            py[:, qc * QCW:(qc + 1) * QCW],
                            wo_sb[:, tp * D + db * DCW: tp * D + (db + 1) * DCW],
                            ot_sb[tp][:, qc * QCW:(qc + 1) * QCW],
                            start=(tp == 0), stop=(tp == 1),
                        )
                y_sb = ysp.tile([128, NQC * QCW], bf16, tag="y", name="y")
                if db % 2 == 0:
                    nc.vector.tensor_copy(y_sb[:], py[:])
                else:
                    _act(nc.scalar, y_sb[:], py[:], AF.Copy)
                nc.sync.dma_start(yt[db * DCW:(db + 1) * DCW, :], y_sb[:])

    nc.compile()
    return nc


def _pack_ndc(w_g):
    """[HD, D] row-slice of a Linear weight -> [128, NDC*HD] SBUF image with
    w[p, c*HD+n] = w_g[n, c*128+p] (lhsT chunks along the free dim)."""
    return np.ascontiguousarray(
        w_g.reshape(HD, NDC, 128).transpose(2, 1, 0).reshape(128, NDC * HD)
    )


def _pack_x(xT):
    """[D, L] activation-transpose -> [128, NQC*NDC*512] with block (qc, c) =
    xT[c*128:(c+1)*128, qc*512:(qc+1)*512], so phase B/C DMAs are contiguous."""
    # xT [D=NDC*128, L=NQC*512] -> [NDC, 128, NQC, 512] -> [128, NQC, NDC, 512]
    return np.ascontiguousarray(
        xT.reshape(NDC, 128, NQC, QCW).transpose(1, 2, 0, 3).reshape(128, NQC * NDC * QCW)
    ).astype(BF)


def _prep_inmaps(query, key, value, Wq, bq, Wk, bk, Wv, bv, Wo, masksize):
    half = int(masksize) // 2
    slots = _band_slots(half)
    ns = max(len(slots), 1)
    em = np.ones((128, ns * 512), np.float32)
    e1 = np.float32(np.exp(np.float32(1.0)))
    p = np.arange(128)[:, None]
    f = np.arange(512)[None, :]
    for d, (si, _, _) in slots.items():
        em[:, si * 512:(si + 1) * 512] = np.where(
            np.abs(d + p - f) <= half, e1, np.float32(1.0)
        )
    em = em.astype(BF)

    xqP = [_pack_x(np.ascontiguousarray(query[b].T)) for b in range(B)]
    xkP = [_pack_x(np.ascontiguousarray(key[b].T)) for b in range(B)]
    xvP = [_pack_x(np.ascontiguousarray(value[b].T)) for b in range(B)]
    wqP = [_pack_ndc(Wq[g * HD:(g + 1) * HD, :]).astype(BF) for g in range(4)]
    wkP = [_pack_ndc(Wk[g * HD:(g + 1) * HD, :]).astype(BF) for g in range(4)]
    wvP = [_pack_ndc(Wv[g * HD:(g + 1) * HD, :]).astype(BF) for g in range(4)]
    # wo: per head-PAIR tp, [128, D] = Wo[:, g*HD+tp*128 : +128].T, laid side
    # by side -> [128, 2*D]
    woP = [
        np.ascontiguousarray(
            np.concatenate(
                [Wo[:, g * HD + tp * 128: g * HD + (tp + 1) * 128].T for tp in range(2)],
                axis=1,
            )
        ).astype(BF)
        for g in range(4)
    ]
    bqP = [np.ascontiguousarray(bq[g * HD:(g + 1) * HD].reshape(2, 128).T) for g in range(4)]
    # bk: column h holds head h's bias in rows (h%2)*64..+64 (rest unused)
    bkP = []
    for g in range(4):
        bz = np.zeros((128, 4), np.float32)
        for h in range(HPC):
            po = (h % 2) * 64
            bz[po:po + 64, h] = bk[g * HD + h * 64: g * HD + (h + 1) * 64]
        bkP.append(bz)
    bvP = [
        np.ascontiguousarray(
            np.concatenate(
                [np.tile(bv[g * HD:(g + 1) * HD], (128, 1)), np.ones((128, 2), np.float32)],
                axis=1,
            )
        )
        for g in range(4)
    ]

    on1z = np.zeros((128, 64), BF)
    on1z[0, :] = 1.0

    in_maps = []
    for c in range(8):
        b, g = c // 4, c % 4
        in_maps.append({
            "xq": xqP[b], "xk": xkP[b], "xv": xvP[b],
            "wq": wqP[g], "wk": wkP[g], "wv": wvP[g], "wo": woP[g],
            "bq": bqP[g], "bk": bkP[g], "bv": bvP[g], "em": em,
            "on1": on1z,
        })
    return in_maps


def kernel(query, key, value, Wq, bq, Wk, bk, Wv, bv, Wo, bo, masksize):
    query = np.asarray(query, dtype=np.float32)
    key = np.asarray(key, dtype=np.float32)
    value = np.asarray(value, dtype=np.float32)
    Wq, bq = np.asarray(Wq, np.float32), np.asarray(bq, np.float32)
    Wk, bk = np.asarray(Wk, np.float32), np.asarray(bk, np.float32)
    Wv, bv = np.asarray(Wv, np.float32), np.asarray(bv, np.float32)
    Wo, bo = np.asarray(Wo, np.float32), np.asarray(bo, np.float32)
    ms = int(np.asarray(masksize))

    if ms not in _CACHE:
        _CACHE[ms] = _build(ms)
    nc = _CACHE[ms]

    in_maps = _prep_inmaps(query, key, value, Wq, bq, Wk, bk, Wv, bv, Wo, ms)
    res = run_bass_kernel_spmd(nc, in_maps, list(range(8)))

    out = np.empty((B, L, D), np.float32)
    for b in range(B):
        acc = res.results[4 * b]["yt"].astype(np.float32)
        for g in range(1, 4):
            acc = acc + res.results[4 * b + g]["yt"].astype(np.float32)
        out[b] = acc.T + bo
    return out
